# revision 26
# baseline (speedup 1.0000x reference)
"""Trainium2 Bass kernel for nn_BitwiseTasNetBlock.

Model: 4 layers of [1x1 conv C->D, PReLU, BN, dilated depthwise conv K=3,
PReLU, BN, 1x1 conv D->C] with a residual around the whole stack.
B=8, C=128, D=512, T=8000. Training-mode BatchNorm -> stats over (batch, time).

Sharding: data-parallel over batch, one batch element per NeuronCore (8 cores).
Each BN needs global (B,T) channel stats: each core computes local per-channel
(sum, sumsq) -- the sum rides for free on the PReLU activation's accum_out;
the sumsq is a per-supertile DVE scalar_tensor_tensor square+accumulate
(one group per BN1 goes to the scalar engine as Square+accum to balance
engines). A small [128, 8] fp32 AllReduce sums (S, Q) across cores and each
core reduces to the global affine (s, t). The last group's final supertile is
split in half so its sumsq pipelines behind the PReLU and the collective
starts sooner. The BN affine is folded into neighbouring ops so no extra
full-tensor pass is needed:
  - BN1 folds into the PReLU2 activation (scale/bias APs); depthwise-conv edge
    columns (where zero-padding makes the folded bias wrong) are fixed with
    per-edge bias variants on the first/last `dilation` columns.
  - BN2 folds into the conv2 weights (scaled on device) + bias (W2 @ t2 via a
    tiny matvec matmul).
All activations/weights are fp16 (PE: 1 cycle/row vs fp32's 4, and 10-bit
mantissa keeps rel err ~1e-3); PSUM accumulation is fp32. Depthwise conv runs
on the tensor engine as diagonal-matrix matmuls (3 taps accumulated in PSUM).
The residual is added in fp32 during the last conv2 psum drain (a fused
scalar_tensor_tensor: out = psum + b2p + x, reading the intact fp32 x stage),
chunked so the output DMAs overlap the adds.
"""

import numpy as np
import ml_dtypes
from contextlib import ExitStack

import concourse.bass as bass
import concourse.bacc as bacc
import concourse.mybir as mybir
import concourse.tile as tile
from concourse.bass_utils import run_bass_kernel_spmd

F32 = mybir.dt.float32
BF16 = mybir.dt.float16  # 16-bit dtype for acts/weights (fp16: 10-bit mantissa)
AF = mybir.ActivationFunctionType
ALU = mybir.AluOpType

NCORES = 8
B, C, D, T, L, K = 8, 128, 512, 8000, 4, 3
G = D // 128          # 4 channel groups of 128 partitions
PAD = 8               # max dilation
W = T + 2 * PAD       # padded activation width
NTW = 512             # matmul free-dim tile (one PSUM bank of f32)
STW = 2048            # psum super-tile (4 banks)
EPS = 1e-5

# 8000 = 3*2048 + 1856: super-tiles of unequal width; use explicit col ranges.
ST_COLS = [(0, 2048), (2048, 4096), (4096, 6144), (6144, 8000)]
NST = len(ST_COLS)    # 4 super-tiles per group

VEC_TABLES = ["b1", "g1", "be1", "bd", "swI", "swL", "swR", "g2", "be2"]
VOFF = {t: j * (L * G) for j, t in enumerate(VEC_TABLES)}

LINEARIZE = False   # total-order scheduling (debug)


def _build_program(alphas1, alphas2):
    nc = bacc.Bacc("TRN2", target_bir_lowering=False, debug=False, num_devices=NCORES)

    xin = nc.dram_tensor("xin", [128, T], F32, kind="ExternalInput")
    w1t = nc.dram_tensor("w1t", [128, L * D], BF16, kind="ExternalInput")
    w2t = nc.dram_tensor("w2t", [128, L * D], BF16, kind="ExternalInput")
    diag = nc.dram_tensor("diag", [128, L * G * K * 128], BF16, kind="ExternalInput")
    vec = nc.dram_tensor("vec", [128, len(VEC_TABLES) * L * G], F32, kind="ExternalInput")
    b2d = nc.dram_tensor("b2d", [128, L], F32, kind="ExternalInput")
    yout = nc.dram_tensor("yout", [128, T], F32, kind="ExternalOutput")

    # collective bounce buffers, one pair per BN
    cins, couts = [], []
    for i in range(L):
        for j in range(2):
            cins.append(nc.dram_tensor(f"cin_{i}_{j}", [128, 2 * G], F32))
            couts.append(
                nc.dram_tensor(
                    f"cout_{i}_{j}", [128, 2 * G], F32, addr_space="Shared"
                )
            )

    rgroups = [list(range(NCORES))]
    warm_in = nc.dram_tensor("warm_in", [128, 2 * G], F32)
    warm_out = nc.dram_tensor("warm_out", [128, 2 * G], F32, addr_space="Shared")

    # Persistent SBUF tensors must stay allocated through TileContext exit:
    # pool slot allocation happens there from the current SBUF heap, so
    # releasing these earlier would let pools overlap their addresses.
    # alloc_sbuf_tensor (no context manager) never frees them.
    with tile.TileContext(nc, linearize=LINEARIZE) as tc, ExitStack() as ctx:
        # ---- persistent SBUF ----
        act = [
            nc.alloc_sbuf_tensor(f"act{j}", [128, W], BF16) for j in range(5)
        ]
        xbf = nc.alloc_sbuf_tensor("xbf", [128, W], BF16)
        xst = nc.alloc_sbuf_tensor("xst", [128, T], F32)   # fp32 x stage / fp32 output
        junk = nc.alloc_sbuf_tensor("junk", [128, STW], BF16)   # DVE sumsq dump
        junkA = nc.alloc_sbuf_tensor("junkA", [128, STW], BF16)  # ACT sumsq dump
        w1s = nc.alloc_sbuf_tensor("w1s", [128, L * D], BF16)
        w2s = nc.alloc_sbuf_tensor("w2s", [128, L * D], BF16)
        vec_s = nc.alloc_sbuf_tensor("vecs", [128, len(VEC_TABLES) * L * G], F32)
        b2_s = nc.alloc_sbuf_tensor("b2s", [128, L], F32)

        psum = ctx.enter_context(tc.tile_pool(name="psum", bufs=2, space="PSUM"))
        small = ctx.enter_context(tc.tile_pool(name="small", bufs=3))
        diagp = ctx.enter_context(tc.tile_pool(name="diagp", bufs=2))

        # ---- initial loads ----
        # input x -> xst (fp32) -> cast to xbf interior, per super-tile so
        # conv1 of layer 0 can start as soon as the first chunk lands.
        # fire a dependency-free warm-up collective immediately: the first
        # real CC otherwise pays a ~22us cold cost on the critical path.
        # (input is uninitialized DRAM; output is never read)
        nc.gpsimd.collective_compute(
            "AllReduce", ALU.add, replica_groups=rgroups,
            ins=[warm_in[:]], outs=[warm_out[:]],
        )
        # also pre-run layer 0's two real collectives on their own buffers
        # (garbage in, output overwritten later) in case part of the cold
        # cost is per descriptor ring
        nc.gpsimd.collective_compute(
            "AllReduce", ALU.add, replica_groups=rgroups,
            ins=[cins[0][:]], outs=[couts[0][:]],
        )
        nc.gpsimd.collective_compute(
            "AllReduce", ALU.add, replica_groups=rgroups,
            ins=[cins[1][:]], outs=[couts[1][:]],
        )
        nc.sync.dma_start(out=xst[:, 0:512], in_=xin[:, 0:512])
        nc.sync.dma_start(out=w1s[:], in_=w1t[:])
        for c0 in range(512, 2048, 512):
            nc.sync.dma_start(out=xst[:, c0 : c0 + 512], in_=xin[:, c0 : c0 + 512])
        # zero the halo pads of every activation slot
        for a in act + [xbf]:
            nc.vector.memset(a[:, 0:PAD], 0.0)
            nc.vector.memset(a[:, PAD + T : W], 0.0)
        for c0 in range(0, 2048, 512):
            nc.scalar.activation(
                out=xbf[:, PAD + c0 : PAD + c0 + 512],
                in_=xst[:, c0 : c0 + 512],
                func=AF.Copy,
            )
        for (c0, c1) in ST_COLS[1:]:
            nc.sync.dma_start(out=xst[:, c0:c1], in_=xin[:, c0:c1])
            nc.scalar.activation(
                out=xbf[:, PAD + c0 : PAD + c1], in_=xst[:, c0:c1], func=AF.Copy
            )
        nc.sync.dma_start(out=w2s[:], in_=w2t[:])
        nc.sync.dma_start(out=vec_s[:], in_=vec[:])
        nc.sync.dma_start(out=b2_s[:], in_=b2d[:])

        def vcol(tbl, i, g=None, n=1):
            off = VOFF[tbl] + i * G + (0 if g is None else g)
            return vec_s[:, off : off + (G if g is None else n)]

        h_idx = None  # layer 0 reads xbf
        for i in range(L):
            delta = 2 ** i
            a1v = float(alphas1[i])
            a2v = float(alphas2[i])
            if i == 0:
                h = xbf
                others = [0, 1, 2, 3, 4]
                p1 = [act[s] for s in others[:4]]
                p2 = [act[s] for s in (4, 0, 1, 2)]
                hn = act[3]
                nh_idx = 3
            else:
                h = act[h_idx]
                others = [s for s in range(5) if s != h_idx]
                p1 = [act[s] for s in others]
                p2 = [act[s] for s in (h_idx, others[0], others[1], others[2])]
                hn = act[others[3]]
                nh_idx = others[3]

            # layer's diagonal depthwise weights
            dg = diagp.tile([128, G * K * 128], BF16, tag="diag")
            nc.sync.dma_start(
                out=dg[:], in_=diag[:, i * G * K * 128 : (i + 1) * G * K * 128]
            )

            # ---- conv1 (C->D) + PReLU1 + local BN1 stats (S via act accum,
            #      Q via per-supertile DVE/GP square+reduce chunks) ----
            pk1 = small.tile([128, 2 * G], F32, tag="pk")  # [S_g0..3 | Q_g0..3]
            for g in range(G):
                lw = w1s[:, (i * G + g) * 128 : (i * G + g + 1) * 128]
                sacc = small.tile([128, NST + 3], F32, tag="sacc")
                qacc = small.tile([128, NST + 1], F32, tag="qacc")
                nsa = nqa = 0
                for st, (s0, s1c) in enumerate(ST_COLS):
                    ps = psum.tile([128, STW], F32, tag="big")
                    for n0 in range(s0, s1c, NTW):
                        n1 = min(n0 + NTW, s1c)
                        nc.tensor.matmul(
                            ps[:, n0 - s0 : n1 - s0],
                            lw,
                            h[:, PAD + n0 : PAD + n1],
                            start=True,
                            stop=True,
                        )
                    # the last group's last supertile gates the collective:
                    # split it in two so its sumsq pipelines behind PReLU
                    if g == G - 1 and st == NST - 1:
                        halves = [(0, 928), (928, s1c - s0)]
                    else:
                        halves = [(0, s1c - s0)]
                    for (e0, e1) in halves:
                        nc.scalar.activation(
                            out=p1[g][:, PAD + s0 + e0 : PAD + s0 + e1],
                            in_=ps[:, e0:e1],
                            func=AF.Prelu,
                            bias=vcol("b1", i, g),
                            scale=1.0,
                            alpha=a1v,
                            accum_out=sacc[:, nsa : nsa + 1],
                        )
                        nsa += 1
                        if g == 0:
                            # balance: one group's sumsq on the scalar engine
                            nc.scalar.activation(
                                out=junkA[:, 0 : e1 - e0],
                                in_=p1[g][:, PAD + s0 + e0 : PAD + s0 + e1],
                                func=AF.Square,
                                accum_out=qacc[:, nqa : nqa + 1],
                            )
                        else:
                            nc.vector.scalar_tensor_tensor(
                                out=junk[:, 0 : e1 - e0],
                                in0=p1[g][:, PAD + s0 + e0 : PAD + s0 + e1],
                                scalar=1.0,
                                in1=p1[g][:, PAD + s0 + e0 : PAD + s0 + e1],
                                op0=ALU.mult,
                                op1=ALU.mult,
                                accum_out=qacc[:, nqa : nqa + 1],
                            )
                        nqa += 1
                nc.vector.tensor_reduce(
                    out=pk1[:, g : g + 1], in_=sacc[:, 0:nsa],
                    axis=mybir.AxisListType.X, op=ALU.add,
                )
                nc.vector.tensor_reduce(
                    out=pk1[:, G + g : G + g + 1], in_=qacc[:, 0:nqa],
                    axis=mybir.AxisListType.X, op=ALU.add,
                )

            # ---- BN1 global stats via AllReduce ----
            s1t, t1t = _emit_cross_stats(
                nc, small, pk1, cins[2 * i], couts[2 * i], rgroups,
                vcol("g1", i), vcol("be1", i),
            )
            biasI = small.tile([128, G], F32, tag="biasI")
            biasL = small.tile([128, G], F32, tag="biasL")
            biasR = small.tile([128, G], F32, tag="biasR")
            for bt, tbl in ((biasI, "swI"), (biasL, "swL"), (biasR, "swR")):
                nc.vector.tensor_mul(bt[:], t1t[:], vcol(tbl, i))
                nc.vector.tensor_add(bt[:], bt[:], vcol("bd", i))

            # ---- depthwise dilated conv (PE diag matmuls) + PReLU2 + stats ----
            pk2 = small.tile([128, 2 * G], F32, tag="pk")
            for g in range(G):
                sacc = small.tile([128, NST + 3], F32, tag="sacc")
                qacc = small.tile([128, NST + 1], F32, tag="qacc")
                nseg = 0
                nqa = 0
                for st, (s0, s1c) in enumerate(ST_COLS):
                    ps = psum.tile([128, STW], F32, tag="big")
                    for k in range(K):
                        off = (k - 1) * delta
                        dw = dg[:, (g * K + k) * 128 : (g * K + k + 1) * 128]
                        for n0 in range(s0, s1c, NTW):
                            n1 = min(n0 + NTW, s1c)
                            nc.tensor.matmul(
                                ps[:, n0 - s0 : n1 - s0],
                                dw,
                                p1[g][:, PAD + n0 + off : PAD + n1 + off],
                                start=(k == 0),
                                stop=(k == K - 1),
                            )
                    # PReLU2 with folded BN1 affine; edge columns use
                    # adjusted biases (zero-padding of the BN output).
                    segs = []
                    if st == 0:
                        segs.append((0, delta, biasL))
                        segs.append((delta, s1c - s0, biasI))
                    elif st == NST - 1:
                        if g == G - 1:
                            # split the gating tail so sumsq pipelines
                            segs.append((0, 928, biasI))
                            segs.append((928, s1c - s0 - delta, biasI))
                        else:
                            segs.append((0, s1c - s0 - delta, biasI))
                        segs.append((s1c - s0 - delta, s1c - s0, biasR))
                    else:
                        segs.append((0, s1c - s0, biasI))
                    for e0, e1, bt in segs:
                        nc.scalar.activation(
                            out=p2[g][:, PAD + s0 + e0 : PAD + s0 + e1],
                            in_=ps[:, e0:e1],
                            func=AF.Prelu,
                            bias=bt[:, g : g + 1],
                            scale=s1t[:, g : g + 1],
                            alpha=a2v,
                            accum_out=sacc[:, nseg : nseg + 1],
                        )
                        nseg += 1
                    if g == G - 1 and st == NST - 1:
                        qh = [(0, 928), (928, s1c - s0)]
                    else:
                        qh = [(0, s1c - s0)]
                    for (e0, e1) in qh:
                        nc.vector.scalar_tensor_tensor(
                            out=junk[:, 0 : e1 - e0],
                            in0=p2[g][:, PAD + s0 + e0 : PAD + s0 + e1],
                            scalar=1.0,
                            in1=p2[g][:, PAD + s0 + e0 : PAD + s0 + e1],
                            op0=ALU.mult,
                            op1=ALU.mult,
                            accum_out=qacc[:, nqa : nqa + 1],
                        )
                        nqa += 1
                nc.vector.tensor_reduce(
                    out=pk2[:, g : g + 1], in_=sacc[:, 0:nseg],
                    axis=mybir.AxisListType.X, op=ALU.add,
                )
                nc.vector.tensor_reduce(
                    out=pk2[:, G + g : G + g + 1], in_=qacc[:, 0:nqa],
                    axis=mybir.AxisListType.X, op=ALU.add,
                )

            # ---- BN2 global stats ----
            s2t, t2t = _emit_cross_stats(
                nc, small, pk2, cins[2 * i + 1], couts[2 * i + 1], rgroups,
                vcol("g2", i), vcol("be2", i),
            )

            # ---- fold BN2 into conv2: scale weights, matvec bias ----
            w2sc = small.tile([128, D], BF16, tag="w2sc")
            for g in range(G):
                nc.vector.tensor_scalar(
                    w2sc[:, g * 128 : (g + 1) * 128],
                    w2s[:, (i * G + g) * 128 : (i * G + g + 1) * 128],
                    s2t[:, g : g + 1],
                    None,
                    ALU.mult,
                )
            t2c = small.tile([128, G], BF16, tag="t2c")
            nc.vector.tensor_scalar(t2c[:], t2t[:], 1.0, None, ALU.mult)
            mvp = psum.tile([128, STW], F32, tag="big")
            for g in range(G):
                nc.tensor.matmul(
                    mvp[:, 0:1],
                    w2s[:, (i * G + g) * 128 : (i * G + g + 1) * 128],
                    t2c[:, g : g + 1],
                    start=(g == 0),
                    stop=(g == G - 1),
                )
            b2p = small.tile([128, 1], F32, tag="b2p")
            nc.vector.tensor_scalar(
                b2p[:], mvp[:, 0:1], b2_s[:, i : i + 1], None, ALU.add
            )

            # ---- conv2 (D->C) [+ residual x via identity matmul on last layer] ----
            last = i == L - 1
            for st, (s0, s1c) in enumerate(ST_COLS):
                ps = psum.tile([128, STW], F32, tag="big")
                for g in range(G):
                    for n0 in range(s0, s1c, NTW):
                        n1 = min(n0 + NTW, s1c)
                        nc.tensor.matmul(
                            ps[:, n0 - s0 : n1 - s0],
                            w2sc[:, g * 128 : (g + 1) * 128],
                            p2[g][:, PAD + n0 : PAD + n1],
                            start=(g == 0),
                            stop=(g == G - 1),
                        )
                if last:
                    # residual + bias fused into the psum drain: out = (ps +
                    # b2p) + x, with x read from the still-intact fp32 stage.
                    # Chunked so the last output DMA overlaps the DVE adds.
                    for e0 in range(s0, s1c, 1024):
                        e1 = min(e0 + 1024, s1c)
                        nc.vector.scalar_tensor_tensor(
                            out=xst[:, e0:e1],
                            in0=ps[:, e0 - s0 : e1 - s0],
                            scalar=b2p[:],
                            in1=xst[:, e0:e1],
                            op0=ALU.add,
                            op1=ALU.add,
                        )
                        nc.sync.dma_start(out=yout[:, e0:e1], in_=xst[:, e0:e1])
                else:
                    nc.vector.tensor_scalar(
                        hn[:, PAD + s0 : PAD + s1c], ps[:, 0 : s1c - s0], b2p[:],
                        None, ALU.add,
                    )

            h_idx = nh_idx

    nc.finalize()
    return nc


def _emit_cross_stats(nc, small, pk, cin, cout, rgroups, gamma, beta):
    """AllReduce per-core (S, Q) channel sums and produce the global BN affine.

    pk: [128, 2G] tile, cols [0:G] = per-group sum, [G:2G] = per-group sumsq
    (each over this core's T columns).
    Returns (s, t) tiles [128, G]: s = gamma*rsqrt(var_g+eps),
    t = beta - mean_g*s.
    """
    Gg = G
    nc.sync.dma_start(out=cin[:], in_=pk[:])
    nc.gpsimd.collective_compute(
        "AllReduce", ALU.add, replica_groups=rgroups, ins=[cin[:]], outs=[cout[:]]
    )
    red = small.tile([128, 2 * Gg], F32, tag="red")
    nc.sync.dma_start(out=red[:], in_=cout[:])
    cnt = 1.0 / (NCORES * T)
    # var + eps = cnt*Q - cnt^2*S^2 + eps, computed in 3 fused DVE ops
    A = small.tile([128, Gg], F32, tag="A")
    nc.vector.tensor_mul(A[:], red[:, 0:Gg], red[:, 0:Gg])          # S^2
    nc.vector.tensor_scalar(A[:], A[:], -cnt * cnt, EPS, ALU.mult, ALU.add)
    ve = small.tile([128, Gg], F32, tag="ve")
    nc.vector.scalar_tensor_tensor(
        out=ve[:], in0=red[:, Gg : 2 * Gg], scalar=cnt, in1=A[:],
        op0=ALU.mult, op1=ALU.add,
    )
    sd = small.tile([128, Gg], F32, tag="sd")
    nc.scalar.activation(out=sd[:], in_=ve[:], func=AF.Sqrt)
    rstd = small.tile([128, Gg], F32, tag="rstd")
    nc.vector.reciprocal(out=rstd[:], in_=sd[:])
    s = small.tile([128, Gg], F32, tag="s")
    nc.vector.tensor_mul(s[:], gamma, rstd[:])
    t = small.tile([128, Gg], F32, tag="t")
    nc.vector.scalar_tensor_tensor(
        out=t[:], in0=red[:, 0:Gg], scalar=cnt, in1=s[:],
        op0=ALU.mult, op1=ALU.mult,
    )  # mean * s
    nc.vector.tensor_sub(t[:], beta, t[:])
    return s, t


_CACHE = {}


def _get_program(a1, a2):
    key = (tuple(np.asarray(a1, dtype=np.float64)), tuple(np.asarray(a2, dtype=np.float64)))
    if key not in _CACHE:
        _CACHE[key] = _build_program(np.asarray(a1), np.asarray(a2))
    return _CACHE[key]


def _pack_params(w1, b1, g1, be1, wd, bd, g2, be2, w2, b2):
    w1 = np.asarray(w1, np.float32)
    w2 = np.asarray(w2, np.float32)
    wd = np.asarray(wd, np.float32)

    w1t = np.concatenate([w1[i].T for i in range(L)], axis=1)  # [C, L*D]
    # conv2 lhsT block (i,g): [128, 128] with [p, c] = W2[c, g*128+p]
    w2t = np.concatenate(
        [w2[i].T[g * 128 : (g + 1) * 128] for i in range(L) for g in range(G)],
        axis=1,
    )
    assert w2t.shape == (128, L * D)

    dblocks = []
    for i in range(L):
        for g in range(G):
            for k in range(K):
                dblocks.append(np.diag(wd[i, g * 128 : (g + 1) * 128, k]))
    diag = np.concatenate(dblocks, axis=1).astype(np.float32)

    def pack16(tbl):
        # tbl [L, D] -> [128, L*G] with col i*G+g
        out = np.empty((128, L * G), np.float32)
        for i in range(L):
            for g in range(G):
                out[:, i * G + g] = tbl[i, g * 128 : (g + 1) * 128]
        return out

    sw = wd.sum(axis=2)          # [L, D]
    swL = wd[:, :, 1] + wd[:, :, 2]
    swR = wd[:, :, 0] + wd[:, :, 1]
    tables = {
        "b1": pack16(np.asarray(b1, np.float32)),
        "g1": pack16(np.asarray(g1, np.float32)),
        "be1": pack16(np.asarray(be1, np.float32)),
        "bd": pack16(np.asarray(bd, np.float32)),
        "swI": pack16(sw),
        "swL": pack16(swL),
        "swR": pack16(swR),
        "g2": pack16(np.asarray(g2, np.float32)),
        "be2": pack16(np.asarray(be2, np.float32)),
    }
    vec = np.concatenate([tables[t] for t in VEC_TABLES], axis=1)
    b2d = np.asarray(b2, np.float32).T.copy()  # [128, L]
    f16 = np.float16
    return {
        "w1t": np.ascontiguousarray(w1t).astype(f16),
        "w2t": np.ascontiguousarray(w2t).astype(f16),
        "diag": np.ascontiguousarray(diag).astype(f16),
        "vec": np.ascontiguousarray(vec),
        "b2d": b2d,
    }


def kernel(x, w1, b1, a1, g1, be1, wd, bd, a2, g2, be2, w2, b2, _trace=False):
    x = np.asarray(x, np.float32)
    nc = _get_program(a1, a2)
    params = _pack_params(w1, b1, g1, be1, wd, bd, g2, be2, w2, b2)
    in_maps = [{"xin": np.ascontiguousarray(x[c]), **params} for c in range(NCORES)]
    res = run_bass_kernel_spmd(nc, in_maps, list(range(NCORES)), trace=_trace)
    out = np.stack([res.results[c]["yout"] for c in range(NCORES)], axis=0)
    kernel._last_result = res
    return out.astype(np.float32)


# revision 28
# speedup vs baseline: 1.0185x; 1.0185x over previous
"""Trainium2 Bass kernel for nn_BitwiseTasNetBlock.

Model: 4 layers of [1x1 conv C->D, PReLU, BN, dilated depthwise conv K=3,
PReLU, BN, 1x1 conv D->C] with a residual around the whole stack.
B=8, C=128, D=512, T=8000. Training-mode BatchNorm -> stats over (batch, time).

Sharding: data-parallel over batch, one batch element per NeuronCore (8 cores).
Each BN needs global (B,T) channel stats: each core computes local per-channel
(sum, sumsq) -- the sum rides for free on the PReLU activation's accum_out;
the sumsq is a per-supertile DVE scalar_tensor_tensor square+accumulate
(one group per BN1 goes to the scalar engine as Square+accum to balance
engines). A small [128, 8] fp32 AllReduce sums (S, Q) across cores and each
core reduces to the global affine (s, t). The last group's final supertile is
split in half so its sumsq pipelines behind the PReLU and the collective
starts sooner. The BN affine is folded into neighbouring ops so no extra
full-tensor pass is needed:
  - BN1 folds into the PReLU2 activation (scale/bias APs); depthwise-conv edge
    columns (where zero-padding makes the folded bias wrong) are fixed with
    per-edge bias variants on the first/last `dilation` columns.
  - BN2 folds into the conv2 weights (scaled on device) + bias (W2 @ t2 via a
    tiny matvec matmul).
All activations/weights are fp16 (PE: 1 cycle/row vs fp32's 4, and 10-bit
mantissa keeps rel err ~1e-3); PSUM accumulation is fp32. Depthwise conv runs
on the tensor engine as diagonal-matrix matmuls (3 taps accumulated in PSUM).
The residual is added in fp32 during the last conv2 psum drain (a fused
scalar_tensor_tensor: out = psum + b2p + x, reading the intact fp32 x stage),
chunked so the output DMAs overlap the adds.
"""

import numpy as np
import ml_dtypes
from contextlib import ExitStack

import concourse.bass as bass
import concourse.bacc as bacc
import concourse.mybir as mybir
import concourse.tile as tile
from concourse.bass_utils import run_bass_kernel_spmd

F32 = mybir.dt.float32
BF16 = mybir.dt.float16  # 16-bit dtype for acts/weights (fp16: 10-bit mantissa)
AF = mybir.ActivationFunctionType
ALU = mybir.AluOpType

NCORES = 8
B, C, D, T, L, K = 8, 128, 512, 8000, 4, 3
G = D // 128          # 4 channel groups of 128 partitions
PAD = 8               # max dilation
W = T + 2 * PAD       # padded activation width
NTW = 512             # matmul free-dim tile (one PSUM bank of f32)
STW = 2048            # psum super-tile (4 banks)
EPS = 1e-5

# 8000 = 3*2048 + 1856: super-tiles of unequal width; use explicit col ranges.
ST_COLS = [(0, 2048), (2048, 4096), (4096, 6144), (6144, 8000)]
NST = len(ST_COLS)    # 4 super-tiles per group

VEC_TABLES = ["b1", "g1", "be1", "bd", "swI", "swL", "swR", "g2", "be2"]
VOFF = {t: j * (L * G) for j, t in enumerate(VEC_TABLES)}

LINEARIZE = False   # total-order scheduling (debug)


def _build_program(alphas1, alphas2):
    nc = bacc.Bacc("TRN2", target_bir_lowering=False, debug=False, num_devices=NCORES)

    xin = nc.dram_tensor("xin", [128, T], F32, kind="ExternalInput")
    w1t = nc.dram_tensor("w1t", [128, L * D], BF16, kind="ExternalInput")
    w2t = nc.dram_tensor("w2t", [128, L * D], BF16, kind="ExternalInput")
    diag = nc.dram_tensor("diag", [128, L * G * K * 128], BF16, kind="ExternalInput")
    vec = nc.dram_tensor("vec", [128, len(VEC_TABLES) * L * G], F32, kind="ExternalInput")
    b2d = nc.dram_tensor("b2d", [128, L], F32, kind="ExternalInput")
    yout = nc.dram_tensor("yout", [128, T], F32, kind="ExternalOutput")

    # collective bounce buffers, one pair per BN
    cins, couts = [], []
    for i in range(L):
        for j in range(2):
            cins.append(nc.dram_tensor(f"cin_{i}_{j}", [128, 2 * G], F32))
            couts.append(
                nc.dram_tensor(
                    f"cout_{i}_{j}", [128, 2 * G], F32, addr_space="Shared"
                )
            )

    rgroups = [list(range(NCORES))]
    warm_in = nc.dram_tensor("warm_in", [128, 2 * G], F32)
    warm_out = nc.dram_tensor("warm_out", [128, 2 * G], F32, addr_space="Shared")

    # Persistent SBUF tensors must stay allocated through TileContext exit:
    # pool slot allocation happens there from the current SBUF heap, so
    # releasing these earlier would let pools overlap their addresses.
    # alloc_sbuf_tensor (no context manager) never frees them.
    with tile.TileContext(nc, linearize=LINEARIZE) as tc, ExitStack() as ctx:
        # ---- persistent SBUF ----
        act = [
            nc.alloc_sbuf_tensor(f"act{j}", [128, W], BF16) for j in range(5)
        ]
        xbf = nc.alloc_sbuf_tensor("xbf", [128, W], BF16)
        xst = nc.alloc_sbuf_tensor("xst", [128, T], F32)   # fp32 x stage / fp32 output
        junk = nc.alloc_sbuf_tensor("junk", [128, STW], BF16)   # DVE sumsq dump
        junkA = nc.alloc_sbuf_tensor("junkA", [128, STW], BF16)  # ACT sumsq dump
        w1s = nc.alloc_sbuf_tensor("w1s", [128, L * D], BF16)
        w2s = nc.alloc_sbuf_tensor("w2s", [128, L * D], BF16)
        vec_s = nc.alloc_sbuf_tensor("vecs", [128, len(VEC_TABLES) * L * G], F32)
        b2_s = nc.alloc_sbuf_tensor("b2s", [128, L], F32)

        psum = ctx.enter_context(tc.tile_pool(name="psum", bufs=2, space="PSUM"))
        small = ctx.enter_context(tc.tile_pool(name="small", bufs=3))
        diagp = ctx.enter_context(tc.tile_pool(name="diagp", bufs=2))

        # ---- initial loads ----
        # input x -> xst (fp32) -> cast to xbf interior, per super-tile so
        # conv1 of layer 0 can start as soon as the first chunk lands.
        # fire a dependency-free warm-up collective immediately: the first
        # real CC otherwise pays a ~22us cold cost on the critical path.
        # (input is uninitialized DRAM; output is never read)
        nc.gpsimd.collective_compute(
            "AllReduce", ALU.add, replica_groups=rgroups,
            ins=[warm_in[:]], outs=[warm_out[:]],
        )
        nc.sync.dma_start(out=xst[:, 0:512], in_=xin[:, 0:512])
        nc.sync.dma_start(out=w1s[:], in_=w1t[:])
        for c0 in range(512, 2048, 512):
            nc.sync.dma_start(out=xst[:, c0 : c0 + 512], in_=xin[:, c0 : c0 + 512])
        # zero the halo pads of every activation slot
        for a in act + [xbf]:
            nc.vector.memset(a[:, 0:PAD], 0.0)
            nc.vector.memset(a[:, PAD + T : W], 0.0)
        for c0 in range(0, 2048, 512):
            nc.scalar.activation(
                out=xbf[:, PAD + c0 : PAD + c0 + 512],
                in_=xst[:, c0 : c0 + 512],
                func=AF.Copy,
            )
        for (c0, c1) in ST_COLS[1:]:
            nc.sync.dma_start(out=xst[:, c0:c1], in_=xin[:, c0:c1])
            nc.scalar.activation(
                out=xbf[:, PAD + c0 : PAD + c1], in_=xst[:, c0:c1], func=AF.Copy
            )
        nc.sync.dma_start(out=w2s[:], in_=w2t[:])
        nc.sync.dma_start(out=vec_s[:], in_=vec[:])
        nc.sync.dma_start(out=b2_s[:], in_=b2d[:])

        def vcol(tbl, i, g=None, n=1):
            off = VOFF[tbl] + i * G + (0 if g is None else g)
            return vec_s[:, off : off + (G if g is None else n)]

        h_idx = None  # layer 0 reads xbf
        for i in range(L):
            delta = 2 ** i
            a1v = float(alphas1[i])
            a2v = float(alphas2[i])
            if i == 0:
                h = xbf
                others = [0, 1, 2, 3, 4]
                p1 = [act[s] for s in others[:4]]
                p2 = [act[s] for s in (4, 0, 1, 2)]
                hn = act[3]
                nh_idx = 3
            else:
                h = act[h_idx]
                others = [s for s in range(5) if s != h_idx]
                p1 = [act[s] for s in others]
                p2 = [act[s] for s in (h_idx, others[0], others[1], others[2])]
                hn = act[others[3]]
                nh_idx = others[3]

            # layer's diagonal depthwise weights
            dg = diagp.tile([128, G * K * 128], BF16, tag="diag")
            nc.sync.dma_start(
                out=dg[:], in_=diag[:, i * G * K * 128 : (i + 1) * G * K * 128]
            )

            # ---- conv1 (C->D) + PReLU1 + local BN1 stats (S via act accum,
            #      Q via per-supertile DVE/GP square+reduce chunks) ----
            pk1 = small.tile([128, 2 * G], F32, tag="pk")  # [S_g0..3 | Q_g0..3]
            for g in range(G):
                lw = w1s[:, (i * G + g) * 128 : (i * G + g + 1) * 128]
                sacc = small.tile([128, NST + 3], F32, tag="sacc")
                qacc = small.tile([128, NST + 1], F32, tag="qacc")
                nsa = nqa = 0
                for st, (s0, s1c) in enumerate(ST_COLS):
                    ps = psum.tile([128, STW], F32, tag="big")
                    for n0 in range(s0, s1c, NTW):
                        n1 = min(n0 + NTW, s1c)
                        nc.tensor.matmul(
                            ps[:, n0 - s0 : n1 - s0],
                            lw,
                            h[:, PAD + n0 : PAD + n1],
                            start=True,
                            stop=True,
                        )
                    # the last group's last supertile gates the collective:
                    # split it in two so its sumsq pipelines behind PReLU
                    if g == G - 1 and st == NST - 1:
                        halves = [(0, 928), (928, s1c - s0)]
                    else:
                        halves = [(0, s1c - s0)]
                    for (e0, e1) in halves:
                        nc.scalar.activation(
                            out=p1[g][:, PAD + s0 + e0 : PAD + s0 + e1],
                            in_=ps[:, e0:e1],
                            func=AF.Prelu,
                            bias=vcol("b1", i, g),
                            scale=1.0,
                            alpha=a1v,
                            accum_out=sacc[:, nsa : nsa + 1],
                        )
                        nsa += 1
                        if g == 0:
                            # balance: one group's sumsq on the scalar engine
                            nc.scalar.activation(
                                out=junkA[:, 0 : e1 - e0],
                                in_=p1[g][:, PAD + s0 + e0 : PAD + s0 + e1],
                                func=AF.Square,
                                accum_out=qacc[:, nqa : nqa + 1],
                            )
                        else:
                            nc.vector.scalar_tensor_tensor(
                                out=junk[:, 0 : e1 - e0],
                                in0=p1[g][:, PAD + s0 + e0 : PAD + s0 + e1],
                                scalar=1.0,
                                in1=p1[g][:, PAD + s0 + e0 : PAD + s0 + e1],
                                op0=ALU.mult,
                                op1=ALU.mult,
                                accum_out=qacc[:, nqa : nqa + 1],
                            )
                        nqa += 1
                nc.vector.tensor_reduce(
                    out=pk1[:, g : g + 1], in_=sacc[:, 0:nsa],
                    axis=mybir.AxisListType.X, op=ALU.add,
                )
                nc.vector.tensor_reduce(
                    out=pk1[:, G + g : G + g + 1], in_=qacc[:, 0:nqa],
                    axis=mybir.AxisListType.X, op=ALU.add,
                )

            # ---- BN1 global stats via AllReduce ----
            s1t, t1t = _emit_cross_stats(
                nc, small, pk1, cins[2 * i], couts[2 * i], rgroups,
                vcol("g1", i), vcol("be1", i),
            )
            biasI = small.tile([128, G], F32, tag="biasI")
            biasL = small.tile([128, G], F32, tag="biasL")
            biasR = small.tile([128, G], F32, tag="biasR")
            for bt, tbl in ((biasI, "swI"), (biasL, "swL"), (biasR, "swR")):
                nc.vector.tensor_mul(bt[:], t1t[:], vcol(tbl, i))
                nc.vector.tensor_add(bt[:], bt[:], vcol("bd", i))

            # ---- depthwise dilated conv (PE diag matmuls) + PReLU2 + stats ----
            pk2 = small.tile([128, 2 * G], F32, tag="pk")
            for g in range(G):
                sacc = small.tile([128, NST + 3], F32, tag="sacc")
                qacc = small.tile([128, NST + 1], F32, tag="qacc")
                nseg = 0
                nqa = 0
                for st, (s0, s1c) in enumerate(ST_COLS):
                    ps = psum.tile([128, STW], F32, tag="big")
                    for k in range(K):
                        off = (k - 1) * delta
                        dw = dg[:, (g * K + k) * 128 : (g * K + k + 1) * 128]
                        for n0 in range(s0, s1c, NTW):
                            n1 = min(n0 + NTW, s1c)
                            nc.tensor.matmul(
                                ps[:, n0 - s0 : n1 - s0],
                                dw,
                                p1[g][:, PAD + n0 + off : PAD + n1 + off],
                                start=(k == 0),
                                stop=(k == K - 1),
                            )
                    # PReLU2 with folded BN1 affine; edge columns use
                    # adjusted biases (zero-padding of the BN output).
                    segs = []
                    if st == 0:
                        segs.append((0, delta, biasL))
                        segs.append((delta, s1c - s0, biasI))
                    elif st == NST - 1:
                        if g == G - 1:
                            # split the gating tail so sumsq pipelines
                            segs.append((0, 928, biasI))
                            segs.append((928, s1c - s0 - delta, biasI))
                        else:
                            segs.append((0, s1c - s0 - delta, biasI))
                        segs.append((s1c - s0 - delta, s1c - s0, biasR))
                    else:
                        segs.append((0, s1c - s0, biasI))
                    for e0, e1, bt in segs:
                        nc.scalar.activation(
                            out=p2[g][:, PAD + s0 + e0 : PAD + s0 + e1],
                            in_=ps[:, e0:e1],
                            func=AF.Prelu,
                            bias=bt[:, g : g + 1],
                            scale=s1t[:, g : g + 1],
                            alpha=a2v,
                            accum_out=sacc[:, nseg : nseg + 1],
                        )
                        nseg += 1
                    if g == G - 1 and st == NST - 1:
                        qh = [(0, 928), (928, s1c - s0)]
                    else:
                        qh = [(0, s1c - s0)]
                    for (e0, e1) in qh:
                        nc.vector.scalar_tensor_tensor(
                            out=junk[:, 0 : e1 - e0],
                            in0=p2[g][:, PAD + s0 + e0 : PAD + s0 + e1],
                            scalar=1.0,
                            in1=p2[g][:, PAD + s0 + e0 : PAD + s0 + e1],
                            op0=ALU.mult,
                            op1=ALU.mult,
                            accum_out=qacc[:, nqa : nqa + 1],
                        )
                        nqa += 1
                nc.vector.tensor_reduce(
                    out=pk2[:, g : g + 1], in_=sacc[:, 0:nseg],
                    axis=mybir.AxisListType.X, op=ALU.add,
                )
                nc.vector.tensor_reduce(
                    out=pk2[:, G + g : G + g + 1], in_=qacc[:, 0:nqa],
                    axis=mybir.AxisListType.X, op=ALU.add,
                )

            # ---- BN2 global stats ----
            s2t, t2t = _emit_cross_stats(
                nc, small, pk2, cins[2 * i + 1], couts[2 * i + 1], rgroups,
                vcol("g2", i), vcol("be2", i),
            )

            # ---- fold BN2 into conv2: scale weights, matvec bias ----
            w2sc = small.tile([128, D], BF16, tag="w2sc")
            for g in range(G):
                nc.vector.tensor_scalar(
                    w2sc[:, g * 128 : (g + 1) * 128],
                    w2s[:, (i * G + g) * 128 : (i * G + g + 1) * 128],
                    s2t[:, g : g + 1],
                    None,
                    ALU.mult,
                )
            t2c = small.tile([128, G], BF16, tag="t2c")
            nc.vector.tensor_scalar(t2c[:], t2t[:], 1.0, None, ALU.mult)
            mvp = psum.tile([128, STW], F32, tag="big")
            for g in range(G):
                nc.tensor.matmul(
                    mvp[:, 0:1],
                    w2s[:, (i * G + g) * 128 : (i * G + g + 1) * 128],
                    t2c[:, g : g + 1],
                    start=(g == 0),
                    stop=(g == G - 1),
                )
            b2p = small.tile([128, 1], F32, tag="b2p")
            nc.vector.tensor_scalar(
                b2p[:], mvp[:, 0:1], b2_s[:, i : i + 1], None, ALU.add
            )

            # ---- conv2 (D->C) [+ residual x via identity matmul on last layer] ----
            last = i == L - 1
            for st, (s0, s1c) in enumerate(ST_COLS):
                ps = psum.tile([128, STW], F32, tag="big")
                for g in range(G):
                    for n0 in range(s0, s1c, NTW):
                        n1 = min(n0 + NTW, s1c)
                        nc.tensor.matmul(
                            ps[:, n0 - s0 : n1 - s0],
                            w2sc[:, g * 128 : (g + 1) * 128],
                            p2[g][:, PAD + n0 : PAD + n1],
                            start=(g == 0),
                            stop=(g == G - 1),
                        )
                if last:
                    # residual + bias fused into the psum drain: out = (ps +
                    # b2p) + x, with x read from the still-intact fp32 stage.
                    # Chunked so the last output DMA overlaps the DVE adds.
                    for e0 in range(s0, s1c, 1024):
                        e1 = min(e0 + 1024, s1c)
                        nc.vector.scalar_tensor_tensor(
                            out=xst[:, e0:e1],
                            in0=ps[:, e0 - s0 : e1 - s0],
                            scalar=b2p[:],
                            in1=xst[:, e0:e1],
                            op0=ALU.add,
                            op1=ALU.add,
                        )
                        nc.sync.dma_start(out=yout[:, e0:e1], in_=xst[:, e0:e1])
                else:
                    # supertile 0 drains in 512-col pieces: the next layer's
                    # first conv1 matmul starts as soon as hn[0:512] lands
                    step = NTW if st == 0 else s1c - s0
                    for e0 in range(0, s1c - s0, step):
                        e1 = min(e0 + step, s1c - s0)
                        nc.vector.tensor_scalar(
                            hn[:, PAD + s0 + e0 : PAD + s0 + e1],
                            ps[:, e0:e1], b2p[:], None, ALU.add,
                        )

            h_idx = nh_idx

    nc.finalize()
    return nc


def _emit_cross_stats(nc, small, pk, cin, cout, rgroups, gamma, beta):
    """AllReduce per-core (S, Q) channel sums and produce the global BN affine.

    pk: [128, 2G] tile, cols [0:G] = per-group sum, [G:2G] = per-group sumsq
    (each over this core's T columns).
    Returns (s, t) tiles [128, G]: s = gamma*rsqrt(var_g+eps),
    t = beta - mean_g*s.
    """
    Gg = G
    nc.sync.dma_start(out=cin[:], in_=pk[:])
    nc.gpsimd.collective_compute(
        "AllReduce", ALU.add, replica_groups=rgroups, ins=[cin[:]], outs=[cout[:]]
    )
    red = small.tile([128, 2 * Gg], F32, tag="red")
    nc.sync.dma_start(out=red[:], in_=cout[:])
    cnt = 1.0 / (NCORES * T)
    # var + eps = cnt*Q - cnt^2*S^2 + eps, computed in 3 fused DVE ops
    A = small.tile([128, Gg], F32, tag="A")
    nc.vector.tensor_mul(A[:], red[:, 0:Gg], red[:, 0:Gg])          # S^2
    nc.vector.tensor_scalar(A[:], A[:], -cnt * cnt, EPS, ALU.mult, ALU.add)
    ve = small.tile([128, Gg], F32, tag="ve")
    nc.vector.scalar_tensor_tensor(
        out=ve[:], in0=red[:, Gg : 2 * Gg], scalar=cnt, in1=A[:],
        op0=ALU.mult, op1=ALU.add,
    )
    sd = small.tile([128, Gg], F32, tag="sd")
    nc.scalar.activation(out=sd[:], in_=ve[:], func=AF.Sqrt)
    rstd = small.tile([128, Gg], F32, tag="rstd")
    nc.vector.reciprocal(out=rstd[:], in_=sd[:])
    s = small.tile([128, Gg], F32, tag="s")
    nc.vector.tensor_mul(s[:], gamma, rstd[:])
    t = small.tile([128, Gg], F32, tag="t")
    nc.vector.scalar_tensor_tensor(
        out=t[:], in0=red[:, 0:Gg], scalar=cnt, in1=s[:],
        op0=ALU.mult, op1=ALU.mult,
    )  # mean * s
    nc.vector.tensor_sub(t[:], beta, t[:])
    return s, t


_CACHE = {}


def _get_program(a1, a2):
    key = (tuple(np.asarray(a1, dtype=np.float64)), tuple(np.asarray(a2, dtype=np.float64)))
    if key not in _CACHE:
        _CACHE[key] = _build_program(np.asarray(a1), np.asarray(a2))
    return _CACHE[key]


def _pack_params(w1, b1, g1, be1, wd, bd, g2, be2, w2, b2):
    w1 = np.asarray(w1, np.float32)
    w2 = np.asarray(w2, np.float32)
    wd = np.asarray(wd, np.float32)

    w1t = np.concatenate([w1[i].T for i in range(L)], axis=1)  # [C, L*D]
    # conv2 lhsT block (i,g): [128, 128] with [p, c] = W2[c, g*128+p]
    w2t = np.concatenate(
        [w2[i].T[g * 128 : (g + 1) * 128] for i in range(L) for g in range(G)],
        axis=1,
    )
    assert w2t.shape == (128, L * D)

    dblocks = []
    for i in range(L):
        for g in range(G):
            for k in range(K):
                dblocks.append(np.diag(wd[i, g * 128 : (g + 1) * 128, k]))
    diag = np.concatenate(dblocks, axis=1).astype(np.float32)

    def pack16(tbl):
        # tbl [L, D] -> [128, L*G] with col i*G+g
        out = np.empty((128, L * G), np.float32)
        for i in range(L):
            for g in range(G):
                out[:, i * G + g] = tbl[i, g * 128 : (g + 1) * 128]
        return out

    sw = wd.sum(axis=2)          # [L, D]
    swL = wd[:, :, 1] + wd[:, :, 2]
    swR = wd[:, :, 0] + wd[:, :, 1]
    tables = {
        "b1": pack16(np.asarray(b1, np.float32)),
        "g1": pack16(np.asarray(g1, np.float32)),
        "be1": pack16(np.asarray(be1, np.float32)),
        "bd": pack16(np.asarray(bd, np.float32)),
        "swI": pack16(sw),
        "swL": pack16(swL),
        "swR": pack16(swR),
        "g2": pack16(np.asarray(g2, np.float32)),
        "be2": pack16(np.asarray(be2, np.float32)),
    }
    vec = np.concatenate([tables[t] for t in VEC_TABLES], axis=1)
    b2d = np.asarray(b2, np.float32).T.copy()  # [128, L]
    f16 = np.float16
    return {
        "w1t": np.ascontiguousarray(w1t).astype(f16),
        "w2t": np.ascontiguousarray(w2t).astype(f16),
        "diag": np.ascontiguousarray(diag).astype(f16),
        "vec": np.ascontiguousarray(vec),
        "b2d": b2d,
    }


def kernel(x, w1, b1, a1, g1, be1, wd, bd, a2, g2, be2, w2, b2, _trace=False):
    x = np.asarray(x, np.float32)
    nc = _get_program(a1, a2)
    params = _pack_params(w1, b1, g1, be1, wd, bd, g2, be2, w2, b2)
    in_maps = [{"xin": np.ascontiguousarray(x[c]), **params} for c in range(NCORES)]
    res = run_bass_kernel_spmd(nc, in_maps, list(range(NCORES)), trace=_trace)
    out = np.stack([res.results[c]["yout"] for c in range(NCORES)], axis=0)
    kernel._last_result = res
    return out.astype(np.float32)
